# revision 12
# baseline (speedup 1.0000x reference)
"""Trainium2 Bass kernel for nn_CrossAttention (linear/efficient attention).

Math: out = x + reshape( x_flat @ W_eff + bo ) where
  W_eff = I + sum_h Wq_h @ cm_h @ Wo_h,
  cm_h  = softmax_n(k_h)^T @ v_h,  k = ctx_flat @ Wk, v = ctx_flat @ Wv.
(The q projection folds into W_eff; the residual folds in as the identity.)

Sharding: 8 cores = 4 batches x 2 token-halves. Each core computes partial
[num|den] softmax statistics over its 8192 tokens; a pairwise AllReduce
merges them; each core then applies W_eff to its own token half.

Precision plan: ctx / Wk / Wv are host-cast to fp8 e4m3 (weights scaled by
16 so they sit in the normal range; the exp activation un-scales with
scale=1/16 and the den column uses 16.0 to keep the num/den ratio exact).
Phase-1 matmuls run in DoubleRow fp8 (K=256 per instruction, 2x rate).
x is host-cast fp16 (feeds phase 2 and the folded residual); output is
stored fp16 and upcast on host. All statistics accumulate in fp32 PSUM.
"""

import sys

if "/opt/trn_rl_repo" not in sys.path:
    sys.path.insert(0, "/opt/trn_rl_repo")

import numpy as np

B = 4
C = 256          # channels (DIM)
N_FULL = 16384   # tokens per batch (128*128)
T = 8192         # tokens per core
HEADS = 8
DH = 64
INNER = 512
NCORES = 8
CHUNK = 512
NCH = T // CHUNK      # 16
SUBS = CHUNK // 128   # 4
PAIRS = SUBS // 2     # 2 token-subtile pairs per chunk (DoubleRow granularity)

_CACHE: dict = {}
LAST_RESULTS = None   # BassKernelResults of the most recent run (for profiling)
TRACE = False         # set True before calling kernel() to capture a trace


def _build_nc():
    import concourse.mybir as mybir
    import concourse.tile as tile
    from concourse import bacc
    from concourse.masks import make_identity

    f32, f16, f8 = mybir.dt.float32, mybir.dt.float16, mybir.dt.float8e4
    AF = mybir.ActivationFunctionType
    DR = mybir.MatmulPerfMode.DoubleRow

    nc = bacc.Bacc("TRN2", target_bir_lowering=False, debug=False)

    xh = nc.dram_tensor("xh", [C, T], f16, kind="ExternalInput")
    ch = nc.dram_tensor("ch", [C, T], f8, kind="ExternalInput")
    wk = nc.dram_tensor("wk", [C, INNER], f8, kind="ExternalInput")
    wv = nc.dram_tensor("wv", [C, INNER], f8, kind="ExternalInput")
    wqt = nc.dram_tensor("wqt", [INNER, C], f16, kind="ExternalInput")
    wo = nc.dram_tensor("wo", [INNER, C], f16, kind="ExternalInput")
    bo = nc.dram_tensor("bo", [C, 1], f32, kind="ExternalInput")
    out = nc.dram_tensor("out", [C, T], f16, kind="ExternalOutput")

    xh_r = xh.ap().rearrange("(kc p) n -> p kc n", p=128)
    ch_r = ch.ap().rearrange("(kc p) n -> p kc n", p=128)
    out_r = out.ap().rearrange("(oc p) n -> p oc n", p=128)

    with tile.TileContext(nc) as tc:
        with (
            tc.tile_pool(name="wpool", bufs=1) as wpool,
            tc.tile_pool(name="spool", bufs=3) as spool,
            tc.tile_pool(name="x16pool", bufs=1) as x16pool,
            tc.tile_pool(name="dpool", bufs=1, space="DRAM") as dpool,
        ):
            # first ctx chunk ahead of everything
            ctx_first = spool.tile([128, 2, CHUNK], f8, tag="ctx", name="ctx0")
            nc.sync.dma_start(ctx_first[:], ch_r[:, :, 0:CHUNK])

            # ---- weights (all host-precast; no on-chip conversions) ----
            wk8 = wpool.tile([128, 2, INNER], f8)
            nc.sync.dma_start(wk8[:], wk.ap().rearrange("(kc p) o -> p kc o", p=128))
            wv8 = wpool.tile([128, 2, INNER], f8)
            nc.sync.dma_start(wv8[:], wv.ap().rearrange("(kc p) o -> p kc o", p=128))
            wqt16 = wpool.tile([128, 4, C], f16)
            nc.sync.dma_start(
                wqt16[:], wqt.ap().rearrange("(hc p) i -> p hc i", p=128)
            )
            wo16 = wpool.tile([64, HEADS, C], f16)
            nc.sync.dma_start(wo16[:], wo.ap().rearrange("(h p) o -> p h o", p=64))
            bo_sb = wpool.tile([128, 2], f32)
            nc.sync.dma_start(bo_sb[:], bo.ap().rearrange("(oc p) x -> p (oc x)", p=128))
            ident16 = wpool.tile([128, 128], f16)
            make_identity(nc, ident16[:])

            # tiny dummy AllReduce issued up front: absorbs the ~11us ncfw
            # wake-up latency so the real collective starts promptly
            dum_in = dpool.tile([1, 2], f32)
            dum_out = dpool.tile([1, 2], f32)
            nc.sync.dma_start(dum_in[:], bo_sb[0:1, 0:2])
            nc.gpsimd.collective_compute(
                "AllReduce",
                mybir.AluOpType.add,
                replica_groups=[[0, 1], [2, 3], [4, 5], [6, 7]],
                ins=[dum_in.opt()],
                outs=[dum_out.opt()],
            )

            # ---- phase 1: accumulate per-head [num | den] over local tokens ----
            # cm_ps[hp] rows 0:64   = head 2hp   : cols 0:64 num, col 64 den
            #           rows 64:128 = head 2hp+1 : cols 65:129 num, col 129 den
            x16_tiles = []

            with (
                tc.tile_pool(name="ps_cm", bufs=1, space="PSUM") as ps_cm,
                tc.tile_pool(name="ps_kv", bufs=1, space="PSUM") as ps_kv,
            ):
                cm_ps = [
                    ps_cm.tile([128, 130], f32, tag=f"cm{i}", name=f"cm{i}")
                    for i in range(4)
                ]
                ctx_next = ctx_first
                pend = None  # (kexp2, vcat2, is_first) awaiting its cm matmuls
                def flush_cm(stop):
                    kexp2, vcat2, first = pend
                    for hp in range(4):
                        nc.tensor.matmul(
                            cm_ps[hp][:],
                            lhsT=kexp2[:, :, hp * 128 : (hp + 1) * 128],
                            rhs=vcat2[:, :, 2 * hp : 2 * hp + 2, :],
                            start=first,
                            stop=stop,
                            perf_mode=DR,
                        )
                for ci in range(NCH):
                    ctx_t = ctx_next
                    if ci + 1 < NCH:
                        ctx_next = spool.tile(
                            [128, 2, CHUNK], f8, tag="ctx", name=f"ctx{ci+1}"
                        )
                        nc.sync.dma_start(
                            ctx_next[:], ch_r[:, :, (ci + 1) * CHUNK : (ci + 2) * CHUNK]
                        )
                    # phase-2 x tile: direct fp16 load, no cast needed
                    x16 = x16pool.tile(
                        [128, 2, CHUNK], f16, tag=f"x16_{ci}", name=f"x16_{ci}"
                    )
                    nc.sync.dma_start(
                        x16[:], xh_r[:, :, ci * CHUNK : (ci + 1) * CHUNK]
                    )
                    x16_tiles.append(x16)

                    for p in range(PAIRS):
                        # two 128-token subtiles per iteration (DoubleRow cm)
                        k2_ps = ps_kv.tile([128, 2, INNER], f32, tag="k2")
                        for s in range(2):
                            tok = slice((2 * p + s) * 128, (2 * p + s + 1) * 128)
                            nc.tensor.matmul(
                                k2_ps[:, s, :],
                                lhsT=ctx_t[:, :, tok],
                                rhs=wk8[:],
                                start=True,
                                stop=True,
                                perf_mode=DR,
                            )
                        # batched exp over both subtiles; un-scale the 16x
                        # weight scaling inside the activation
                        kexp2 = spool.tile([128, 2, INNER], f8, tag="kexp")
                        nc.scalar.activation(kexp2[:], k2_ps[:], AF.Exp, scale=0.0625)
                        v2_ps = ps_kv.tile([128, 2, INNER], f32, tag="v2")
                        for s in range(2):
                            tok = slice((2 * p + s) * 128, (2 * p + s + 1) * 128)
                            nc.tensor.matmul(
                                v2_ps[:, s, :],
                                lhsT=ctx_t[:, :, tok],
                                rhs=wv8[:],
                                start=True,
                                stop=True,
                                perf_mode=DR,
                            )
                        vcat2 = spool.tile([128, 2, 8, 65], f8, tag="vcat")
                        vsrc = v2_ps[:].rearrange("q s (h e) -> q s h e", h=8)
                        nc.vector.tensor_copy(vcat2[:, :, :, 0:64], vsrc)
                        # den column: 16.0 matches the 16x scale baked into wv
                        nc.gpsimd.memset(vcat2[:, :, :, 64], 16.0)
                        # software pipelining: cm matmuls run one pair behind,
                        # so the PE never stalls waiting for this pair's
                        # exp/vcat to drain
                        if pend is not None:
                            flush_cm(False)
                        pend = (kexp2, vcat2, ci == 0 and p == 0)
                flush_cm(True)

                # ---- stage [num|den] for the AllReduce (PSUM -> SBUF -> DRAM) ----
                cm_sb = wpool.tile([128, 4, 65], f32)
                for hp in range(4):
                    nc.vector.tensor_copy(cm_sb[0:64, hp, :], cm_ps[hp][0:64, 0:65])
                    nc.vector.tensor_copy(
                        cm_sb[64:128, hp, :], cm_ps[hp][64:128, 65:130]
                    )
                cc_in = dpool.tile([128, 4, 65], f32)
                nc.sync.dma_start(cc_in[:], cm_sb[:])

            cc_out = dpool.tile([128, 4, 65], f32)
            nc.gpsimd.collective_compute(
                "AllReduce",
                mybir.AluOpType.add,
                replica_groups=[[0, 1], [2, 3], [4, 5], [6, 7]],
                ins=[cc_in.opt()],
                outs=[cc_out.opt()],
            )
            mm_sb = wpool.tile([128, 4, 65], f32)
            nc.sync.dma_start(mm_sb[:], cc_out[:])

            with tc.tile_pool(name="ps_p2", bufs=4, space="PSUM") as ps_p2:
                # keep the PE p-state up through the AllReduce window
                # (fine-grained so real work drains the queue quickly)
                warm_ps = ps_p2.tile([128, INNER], f32, tag="o", name="warm_ps")
                for _ in range(150):
                    nc.tensor.matmul(
                        warm_ps[:, 0:128],
                        lhsT=wqt16[:, 0, 0:128],
                        rhs=wqt16[:, 1, 0:128],
                        start=True,
                        stop=True,
                    )

                # ---- normalize cm, build W_eff = I + sum_h Wq_h cm_h Wo_h ----
                deninv = wpool.tile([128, 4], f32)
                cmn16 = wpool.tile([128, 4, 64], f16)
                m1t16 = wpool.tile([64, 8, C], f16)
                weff16 = wpool.tile([128, 2, C], f16)
                with tc.tile_pool(name="ps_post", bufs=2, space="PSUM") as ps_post:
                    nc.vector.reciprocal(deninv[:], mm_sb[:, :, 64])
                    for hp in range(4):
                        nc.vector.tensor_scalar_mul(
                            cmn16[:, hp, :],
                            mm_sb[:, hp, 0:64],
                            deninv[:, hp : hp + 1],
                        )
                    for h in range(HEADS):
                        hp, hh = h // 2, h % 2
                        rs = slice(hh * 64, hh * 64 + 64)
                        m1t_ps = ps_post.tile([64, C], f32, tag="m1t")
                        nc.tensor.matmul(
                            m1t_ps[:],
                            lhsT=cmn16[rs, hp, :],
                            rhs=wqt16[rs, hp, :],
                            start=True,
                            stop=True,
                        )
                        nc.vector.tensor_copy(m1t16[:, h, :], m1t_ps[:])
                    for ic in range(2):
                        weff_ps = ps_post.tile([128, C], f32, tag="weff")
                        for h in range(HEADS):
                            nc.tensor.matmul(
                                weff_ps[:],
                                lhsT=m1t16[:, h, ic * 128 : (ic + 1) * 128],
                                rhs=wo16[:, h, :],
                                start=(h == 0),
                                stop=False,
                            )
                        # fold the residual in: W_eff += I (this core's row block)
                        nc.tensor.matmul(
                            weff_ps[:, ic * 128 : (ic + 1) * 128],
                            lhsT=ident16[:],
                            rhs=ident16[:],
                            start=False,
                            stop=True,
                        )
                        nc.vector.tensor_copy(weff16[:, ic, :], weff_ps[:])

                # ---- phase 2: out = W_eff^T @ x (+bo), fp16 out ----
                for ci in range(NCH):
                    x16 = x16_tiles[ci]
                    out_sb = spool.tile([128, 2, CHUNK], f16, tag="out_sb")
                    for oc in range(2):
                        o_ps = ps_p2.tile([128, INNER], f32, tag="o", name="o_ps")
                        for ic in range(2):
                            nc.tensor.matmul(
                                o_ps[:, 0:CHUNK],
                                lhsT=weff16[:, ic, oc * 128 : (oc + 1) * 128],
                                rhs=x16[:, ic, :],
                                start=(ic == 0),
                                stop=(ic == 1),
                            )
                        # bias+copy: alternate ACT / DVE to balance engines
                        if (ci + oc) % 2 == 0:
                            nc.scalar.activation(
                                out_sb[:, oc, :],
                                o_ps[:, 0:CHUNK],
                                AF.Identity,
                                bias=bo_sb[:, oc : oc + 1],
                            )
                        else:
                            nc.vector.tensor_scalar_add(
                                out_sb[:, oc, :], o_ps[:, 0:CHUNK], bo_sb[:, oc : oc + 1]
                            )
                    nc.sync.dma_start(
                        out_r[:, :, ci * CHUNK : (ci + 1) * CHUNK], out_sb[:]
                    )

    nc.compile()
    return nc


def _get_nc():
    if "nc" not in _CACHE:
        _CACHE["nc"] = _build_nc()
    return _CACHE["nc"]


def kernel(**inputs) -> np.ndarray:
    global LAST_RESULTS
    import ml_dtypes
    from concourse.bass_utils import run_bass_kernel_spmd

    f8 = ml_dtypes.float8_e4m3fn
    x = np.asarray(inputs["x"], dtype=np.float32)
    ctx = np.asarray(inputs["context"], dtype=np.float32)
    Wq = np.asarray(inputs["Wq"], dtype=np.float32)
    Wk = np.asarray(inputs["Wk"], dtype=np.float32)
    Wv = np.asarray(inputs["Wv"], dtype=np.float32)
    Wo = np.asarray(inputs["Wo"], dtype=np.float32)
    bo = np.ascontiguousarray(
        np.asarray(inputs["bo"], dtype=np.float32).reshape(C, 1)
    )
    wqt = np.ascontiguousarray(Wq.T.astype(np.float16))
    wo16 = np.ascontiguousarray(Wo.astype(np.float16))
    wk8 = np.ascontiguousarray((Wk * 16.0).astype(f8))
    wv8 = np.ascontiguousarray((Wv * 16.0).astype(f8))

    xf = x.reshape(B, C, N_FULL).astype(np.float16)
    cf = ctx.reshape(B, C, N_FULL).astype(f8)

    in_maps = []
    for c in range(NCORES):
        b, t = c // 2, c % 2
        sl = slice(t * T, (t + 1) * T)
        in_maps.append(
            {
                "xh": np.ascontiguousarray(xf[b, :, sl]),
                "ch": np.ascontiguousarray(cf[b, :, sl]),
                "wk": wk8,
                "wv": wv8,
                "wqt": wqt,
                "wo": wo16,
                "bo": bo,
            }
        )

    nc = _get_nc()
    res = run_bass_kernel_spmd(nc, in_maps, list(range(NCORES)), trace=TRACE)
    LAST_RESULTS = res

    out = np.empty((B, C, N_FULL), dtype=np.float32)
    for c in range(NCORES):
        b, t = c // 2, c % 2
        out[b, :, t * T : (t + 1) * T] = res.results[c]["out"].astype(np.float32)
    return out.reshape(B, C, 128, 128)


# revision 13
# speedup vs baseline: 1.1431x; 1.1431x over previous
"""Trainium2 Bass kernel for nn_CrossAttention (linear/efficient attention).

Math: out = x + reshape( x_flat @ W_eff + bo ) where
  W_eff = I + sum_h Wq_h @ cm_h @ Wo_h,
  cm_h  = softmax_n(k_h)^T @ v_h,  k = ctx_flat @ Wk, v = ctx_flat @ Wv.
(The q projection folds into W_eff; the residual folds in as the identity.)

Sharding: 8 cores = 4 batches x 2 token-halves. Each core computes partial
[num|den] softmax statistics over its 8192 tokens; a pairwise AllReduce
merges them; each core then applies W_eff to its own token half.

Precision plan: ctx / Wk / Wv are host-cast to fp8 e4m3 (weights scaled by
16 so they sit in the normal range; the exp activation un-scales with
scale=1/16 and the den column uses 16.0 to keep the num/den ratio exact).
Phase-1 matmuls run in DoubleRow fp8 (K=256 per instruction, 2x rate).
x is host-cast fp16 (feeds phase 2 and the folded residual); output is
stored fp16 and upcast on host. All statistics accumulate in fp32 PSUM.
"""

import sys

if "/opt/trn_rl_repo" not in sys.path:
    sys.path.insert(0, "/opt/trn_rl_repo")

import numpy as np

B = 4
C = 256          # channels (DIM)
N_FULL = 16384   # tokens per batch (128*128)
T = 8192         # tokens per core
HEADS = 8
DH = 64
INNER = 512
NCORES = 8
CHUNK = 512
NCH = T // CHUNK      # 16
SUBS = CHUNK // 128   # 4
PAIRS = SUBS // 2     # 2 token-subtile pairs per chunk (DoubleRow granularity)

_CACHE: dict = {}
LAST_RESULTS = None   # BassKernelResults of the most recent run (for profiling)
TRACE = False         # set True before calling kernel() to capture a trace


def _build_nc():
    import concourse.mybir as mybir
    import concourse.tile as tile
    from concourse import bacc
    from concourse.masks import make_identity

    f32, f16, f8 = mybir.dt.float32, mybir.dt.float16, mybir.dt.float8e4
    AF = mybir.ActivationFunctionType
    DR = mybir.MatmulPerfMode.DoubleRow

    nc = bacc.Bacc("TRN2", target_bir_lowering=False, debug=False)

    xh = nc.dram_tensor("xh", [C, T], f16, kind="ExternalInput")
    ch = nc.dram_tensor("ch", [C, T], f8, kind="ExternalInput")
    wk = nc.dram_tensor("wk", [C, INNER], f8, kind="ExternalInput")
    wv = nc.dram_tensor("wv", [C, INNER], f8, kind="ExternalInput")
    wqt = nc.dram_tensor("wqt", [INNER, C], f16, kind="ExternalInput")
    wo = nc.dram_tensor("wo", [INNER, C], f16, kind="ExternalInput")
    bo = nc.dram_tensor("bo", [C, 1], f32, kind="ExternalInput")
    out = nc.dram_tensor("out", [C, T], f16, kind="ExternalOutput")

    xh_r = xh.ap().rearrange("(kc p) n -> p kc n", p=128)
    ch_r = ch.ap().rearrange("(kc p) n -> p kc n", p=128)
    out_r = out.ap().rearrange("(oc p) n -> p oc n", p=128)

    with tile.TileContext(nc) as tc:
        with (
            tc.tile_pool(name="wpool", bufs=1) as wpool,
            tc.tile_pool(name="spool", bufs=3) as spool,
            tc.tile_pool(name="x16pool", bufs=1) as x16pool,
            tc.tile_pool(name="dpool", bufs=1, space="DRAM") as dpool,
        ):
            # first ctx chunk ahead of everything
            ctx_first = spool.tile([128, 2, CHUNK], f8, tag="ctx", name="ctx0")
            nc.sync.dma_start(ctx_first[:], ch_r[:, :, 0:CHUNK])

            # ---- weights (all host-precast; no on-chip conversions) ----
            wk8 = wpool.tile([128, 2, INNER], f8)
            nc.sync.dma_start(wk8[:], wk.ap().rearrange("(kc p) o -> p kc o", p=128))
            wv8 = wpool.tile([128, 2, INNER], f8)
            nc.sync.dma_start(wv8[:], wv.ap().rearrange("(kc p) o -> p kc o", p=128))
            wqt16 = wpool.tile([128, 4, C], f16)
            nc.sync.dma_start(
                wqt16[:], wqt.ap().rearrange("(hc p) i -> p hc i", p=128)
            )
            wo16 = wpool.tile([64, HEADS, C], f16)
            nc.sync.dma_start(wo16[:], wo.ap().rearrange("(h p) o -> p h o", p=64))
            bo_sb = wpool.tile([128, 2], f32)
            nc.sync.dma_start(bo_sb[:], bo.ap().rearrange("(oc p) x -> p (oc x)", p=128))
            ident16 = wpool.tile([128, 128], f16)
            make_identity(nc, ident16[:])



            # ---- phase 1: accumulate per-head [num | den] over local tokens ----
            # cm_ps[hp] rows 0:64   = head 2hp   : cols 0:64 num, col 64 den
            #           rows 64:128 = head 2hp+1 : cols 65:129 num, col 129 den
            x16_tiles = []

            with (
                tc.tile_pool(name="ps_cm", bufs=1, space="PSUM") as ps_cm,
                tc.tile_pool(name="ps_kv", bufs=1, space="PSUM") as ps_kv,
            ):
                cm_ps = [
                    ps_cm.tile([128, 130], f32, tag=f"cm{i}", name=f"cm{i}")
                    for i in range(4)
                ]
                ctx_next = ctx_first
                pend = None  # (kexp2, vcat2, is_first) awaiting its cm matmuls
                def flush_cm(stop):
                    kexp2, vcat2, first = pend
                    for hp in range(4):
                        nc.tensor.matmul(
                            cm_ps[hp][:],
                            lhsT=kexp2[:, :, hp * 128 : (hp + 1) * 128],
                            rhs=vcat2[:, :, 2 * hp : 2 * hp + 2, :],
                            start=first,
                            stop=stop,
                            perf_mode=DR,
                        )
                for ci in range(NCH):
                    ctx_t = ctx_next
                    if ci + 1 < NCH:
                        ctx_next = spool.tile(
                            [128, 2, CHUNK], f8, tag="ctx", name=f"ctx{ci+1}"
                        )
                        nc.sync.dma_start(
                            ctx_next[:], ch_r[:, :, (ci + 1) * CHUNK : (ci + 2) * CHUNK]
                        )
                    # phase-2 x tile: direct fp16 load, no cast needed
                    x16 = x16pool.tile(
                        [128, 2, CHUNK], f16, tag=f"x16_{ci}", name=f"x16_{ci}"
                    )
                    nc.sync.dma_start(
                        x16[:], xh_r[:, :, ci * CHUNK : (ci + 1) * CHUNK]
                    )
                    x16_tiles.append(x16)

                    for p in range(PAIRS):
                        # two 128-token subtiles per iteration (DoubleRow cm)
                        k2_ps = ps_kv.tile([128, 2, INNER], f32, tag="k2")
                        for s in range(2):
                            tok = slice((2 * p + s) * 128, (2 * p + s + 1) * 128)
                            nc.tensor.matmul(
                                k2_ps[:, s, :],
                                lhsT=ctx_t[:, :, tok],
                                rhs=wk8[:],
                                start=True,
                                stop=True,
                                perf_mode=DR,
                            )
                        # batched exp over both subtiles; un-scale the 16x
                        # weight scaling inside the activation
                        kexp2 = spool.tile([128, 2, INNER], f8, tag="kexp")
                        nc.scalar.activation(kexp2[:], k2_ps[:], AF.Exp, scale=0.0625)
                        v2_ps = ps_kv.tile([128, 2, INNER], f32, tag="v2")
                        for s in range(2):
                            tok = slice((2 * p + s) * 128, (2 * p + s + 1) * 128)
                            nc.tensor.matmul(
                                v2_ps[:, s, :],
                                lhsT=ctx_t[:, :, tok],
                                rhs=wv8[:],
                                start=True,
                                stop=True,
                                perf_mode=DR,
                            )
                        vcat2 = spool.tile([128, 2, 8, 65], f8, tag="vcat")
                        vsrc = v2_ps[:].rearrange("q s (h e) -> q s h e", h=8)
                        nc.vector.tensor_copy(vcat2[:, :, :, 0:64], vsrc)
                        # den column: 16.0 matches the 16x scale baked into wv
                        nc.gpsimd.memset(vcat2[:, :, :, 64], 16.0)
                        # software pipelining: cm matmuls run one pair behind,
                        # so the PE never stalls waiting for this pair's
                        # exp/vcat to drain
                        if pend is not None:
                            flush_cm(False)
                        pend = (kexp2, vcat2, ci == 0 and p == 0)
                flush_cm(True)

                # ---- stage [num|den] for the AllReduce (PSUM -> SBUF -> DRAM) ----
                cm_sb = wpool.tile([128, 4, 65], f32)
                for hp in range(4):
                    nc.vector.tensor_copy(cm_sb[0:64, hp, :], cm_ps[hp][0:64, 0:65])
                    nc.vector.tensor_copy(
                        cm_sb[64:128, hp, :], cm_ps[hp][64:128, 65:130]
                    )
                cc_in = dpool.tile([128, 4, 65], f32)
                nc.sync.dma_start(cc_in[:], cm_sb[:])

            cc_out = dpool.tile([128, 4, 65], f32)
            nc.gpsimd.collective_compute(
                "AllReduce",
                mybir.AluOpType.add,
                replica_groups=[[0, 1], [2, 3], [4, 5], [6, 7]],
                ins=[cc_in.opt()],
                outs=[cc_out.opt()],
            )
            mm_sb = wpool.tile([128, 4, 65], f32)
            nc.sync.dma_start(mm_sb[:], cc_out[:])

            with tc.tile_pool(name="ps_p2", bufs=4, space="PSUM") as ps_p2:
                # keep the PE p-state up through the AllReduce window
                # (fine-grained so real work drains the queue quickly)
                warm_ps = ps_p2.tile([128, INNER], f32, tag="o", name="warm_ps")
                for _ in range(150):
                    nc.tensor.matmul(
                        warm_ps[:, 0:128],
                        lhsT=wqt16[:, 0, 0:128],
                        rhs=wqt16[:, 1, 0:128],
                        start=True,
                        stop=True,
                    )

                # ---- normalize cm, build W_eff = I + sum_h Wq_h cm_h Wo_h ----
                deninv = wpool.tile([128, 4], f32)
                cmn16 = wpool.tile([128, 4, 64], f16)
                m1t16 = wpool.tile([64, 8, C], f16)
                weff16 = wpool.tile([128, 2, C], f16)
                with tc.tile_pool(name="ps_post", bufs=2, space="PSUM") as ps_post:
                    nc.vector.reciprocal(deninv[:], mm_sb[:, :, 64])
                    for hp in range(4):
                        nc.vector.tensor_scalar_mul(
                            cmn16[:, hp, :],
                            mm_sb[:, hp, 0:64],
                            deninv[:, hp : hp + 1],
                        )
                    for h in range(HEADS):
                        hp, hh = h // 2, h % 2
                        rs = slice(hh * 64, hh * 64 + 64)
                        m1t_ps = ps_post.tile([64, C], f32, tag="m1t")
                        nc.tensor.matmul(
                            m1t_ps[:],
                            lhsT=cmn16[rs, hp, :],
                            rhs=wqt16[rs, hp, :],
                            start=True,
                            stop=True,
                        )
                        nc.vector.tensor_copy(m1t16[:, h, :], m1t_ps[:])
                    for ic in range(2):
                        weff_ps = ps_post.tile([128, C], f32, tag="weff")
                        for h in range(HEADS):
                            nc.tensor.matmul(
                                weff_ps[:],
                                lhsT=m1t16[:, h, ic * 128 : (ic + 1) * 128],
                                rhs=wo16[:, h, :],
                                start=(h == 0),
                                stop=False,
                            )
                        # fold the residual in: W_eff += I (this core's row block)
                        nc.tensor.matmul(
                            weff_ps[:, ic * 128 : (ic + 1) * 128],
                            lhsT=ident16[:],
                            rhs=ident16[:],
                            start=False,
                            stop=True,
                        )
                        nc.vector.tensor_copy(weff16[:, ic, :], weff_ps[:])

                # ---- phase 2: out = W_eff^T @ x (+bo), fp16 out ----
                for ci in range(NCH):
                    x16 = x16_tiles[ci]
                    out_sb = spool.tile([128, 2, CHUNK], f16, tag="out_sb")
                    for oc in range(2):
                        o_ps = ps_p2.tile([128, INNER], f32, tag="o", name="o_ps")
                        for ic in range(2):
                            nc.tensor.matmul(
                                o_ps[:, 0:CHUNK],
                                lhsT=weff16[:, ic, oc * 128 : (oc + 1) * 128],
                                rhs=x16[:, ic, :],
                                start=(ic == 0),
                                stop=(ic == 1),
                            )
                        # bias+copy: alternate ACT / DVE to balance engines
                        if (ci + oc) % 2 == 0:
                            nc.scalar.activation(
                                out_sb[:, oc, :],
                                o_ps[:, 0:CHUNK],
                                AF.Identity,
                                bias=bo_sb[:, oc : oc + 1],
                            )
                        else:
                            nc.vector.tensor_scalar_add(
                                out_sb[:, oc, :], o_ps[:, 0:CHUNK], bo_sb[:, oc : oc + 1]
                            )
                    nc.sync.dma_start(
                        out_r[:, :, ci * CHUNK : (ci + 1) * CHUNK], out_sb[:]
                    )

    nc.compile()
    return nc


def _get_nc():
    if "nc" not in _CACHE:
        _CACHE["nc"] = _build_nc()
    return _CACHE["nc"]


def kernel(**inputs) -> np.ndarray:
    global LAST_RESULTS
    import ml_dtypes
    from concourse.bass_utils import run_bass_kernel_spmd

    f8 = ml_dtypes.float8_e4m3fn
    x = np.asarray(inputs["x"], dtype=np.float32)
    ctx = np.asarray(inputs["context"], dtype=np.float32)
    Wq = np.asarray(inputs["Wq"], dtype=np.float32)
    Wk = np.asarray(inputs["Wk"], dtype=np.float32)
    Wv = np.asarray(inputs["Wv"], dtype=np.float32)
    Wo = np.asarray(inputs["Wo"], dtype=np.float32)
    bo = np.ascontiguousarray(
        np.asarray(inputs["bo"], dtype=np.float32).reshape(C, 1)
    )
    wqt = np.ascontiguousarray(Wq.T.astype(np.float16))
    wo16 = np.ascontiguousarray(Wo.astype(np.float16))
    wk8 = np.ascontiguousarray((Wk * 16.0).astype(f8))
    wv8 = np.ascontiguousarray((Wv * 16.0).astype(f8))

    xf = x.reshape(B, C, N_FULL).astype(np.float16)
    cf = ctx.reshape(B, C, N_FULL).astype(f8)

    in_maps = []
    for c in range(NCORES):
        b, t = c // 2, c % 2
        sl = slice(t * T, (t + 1) * T)
        in_maps.append(
            {
                "xh": np.ascontiguousarray(xf[b, :, sl]),
                "ch": np.ascontiguousarray(cf[b, :, sl]),
                "wk": wk8,
                "wv": wv8,
                "wqt": wqt,
                "wo": wo16,
                "bo": bo,
            }
        )

    nc = _get_nc()
    res = run_bass_kernel_spmd(nc, in_maps, list(range(NCORES)), trace=TRACE)
    LAST_RESULTS = res

    out = np.empty((B, C, N_FULL), dtype=np.float32)
    for c in range(NCORES):
        b, t = c // 2, c % 2
        out[b, :, t * T : (t + 1) * T] = res.results[c]["out"].astype(np.float32)
    return out.reshape(B, C, 128, 128)


# revision 22
# speedup vs baseline: 1.1832x; 1.0350x over previous
"""Trainium2 Bass kernel for nn_CrossAttention (linear/efficient attention).

Math: out = x + reshape( x_flat @ W_eff + bo ) where
  W_eff = I + sum_h Wq_h @ cm_h @ Wo_h,
  cm_h  = softmax_n(k_h)^T @ v_h,  k = ctx_flat @ Wk, v = ctx_flat @ Wv.
(The q projection folds into W_eff; the residual folds in as the identity.)

Sharding: 8 cores = 4 batches x 2 token-halves. Each core computes partial
[num|den] softmax statistics over its 8192 tokens; a pairwise AllReduce
merges them; each core then applies W_eff to its own token half.

Precision plan: ctx / Wk / Wv are host-cast to fp8 e4m3 (weights scaled by
16 so they sit in the normal range; the exp activation un-scales with
scale=1/16 and the den column uses 16.0 to keep the num/den ratio exact).
Phase-1 matmuls run in DoubleRow fp8 (K=256 per instruction, 2x rate).
x is host-cast fp16 (feeds phase 2 and the folded residual); output is
stored fp16 and upcast on host. All statistics accumulate in fp32 PSUM.
"""

import sys

if "/opt/trn_rl_repo" not in sys.path:
    sys.path.insert(0, "/opt/trn_rl_repo")

import numpy as np

B = 4
C = 256          # channels (DIM)
N_FULL = 16384   # tokens per batch (128*128)
T = 8192         # tokens per core
HEADS = 8
DH = 64
INNER = 512
NCORES = 8
CHUNK = 512
NCH = T // CHUNK      # 16
SUBS = CHUNK // 128   # 4
PAIRS = SUBS // 2     # 2 token-subtile pairs per chunk (DoubleRow granularity)

_CACHE: dict = {}
LAST_RESULTS = None   # BassKernelResults of the most recent run (for profiling)
TRACE = False         # set True before calling kernel() to capture a trace


def _build_nc():
    import concourse.mybir as mybir
    import concourse.tile as tile
    from concourse import bacc
    from concourse.masks import make_identity

    f32, f16, f8 = mybir.dt.float32, mybir.dt.float16, mybir.dt.float8e4
    AF = mybir.ActivationFunctionType
    DR = mybir.MatmulPerfMode.DoubleRow

    nc = bacc.Bacc("TRN2", target_bir_lowering=False, debug=False)

    xh = nc.dram_tensor("xh", [C, T], f16, kind="ExternalInput")
    ch = nc.dram_tensor("ch", [C, T], f8, kind="ExternalInput")
    wk = nc.dram_tensor("wk", [C, INNER], f8, kind="ExternalInput")
    wv = nc.dram_tensor("wv", [C, INNER], f8, kind="ExternalInput")
    wqt = nc.dram_tensor("wqt", [INNER, C], f16, kind="ExternalInput")
    wo = nc.dram_tensor("wo", [INNER, C], f16, kind="ExternalInput")
    bo = nc.dram_tensor("bo", [C, 1], f32, kind="ExternalInput")
    out = nc.dram_tensor("out", [C, T], f16, kind="ExternalOutput")

    xh_r = xh.ap().rearrange("(kc p) n -> p kc n", p=128)
    ch_r = ch.ap().rearrange("(kc p) n -> p kc n", p=128)
    out_r = out.ap().rearrange("(oc p) n -> p oc n", p=128)

    with tile.TileContext(nc) as tc:
        with (
            tc.tile_pool(name="wpool", bufs=1) as wpool,
            tc.tile_pool(name="spool", bufs=3) as spool,
            tc.tile_pool(name="x16pool", bufs=1) as x16pool,
            tc.tile_pool(name="dpool", bufs=1, space="DRAM") as dpool,
        ):
            # first ctx chunk ahead of everything
            ctx_first = spool.tile([128, 2, CHUNK], f8, tag="ctx", name="ctx0")
            nc.sync.dma_start(ctx_first[:], ch_r[:, :, 0:CHUNK])

            # ---- weights (all host-precast; no on-chip conversions) ----
            wk8 = wpool.tile([128, 2, INNER], f8)
            nc.sync.dma_start(wk8[:], wk.ap().rearrange("(kc p) o -> p kc o", p=128))
            wv8 = wpool.tile([128, 2, INNER], f8)
            nc.sync.dma_start(wv8[:], wv.ap().rearrange("(kc p) o -> p kc o", p=128))
            wqt16 = wpool.tile([128, 4, C], f16)
            nc.sync.dma_start(
                wqt16[:], wqt.ap().rearrange("(hc p) i -> p hc i", p=128)
            )
            wo16 = wpool.tile([64, HEADS, C], f16)
            nc.sync.dma_start(wo16[:], wo.ap().rearrange("(h p) o -> p h o", p=64))
            bo_sb = wpool.tile([128, 2], f32)
            nc.sync.dma_start(bo_sb[:], bo.ap().rearrange("(oc p) x -> p (oc x)", p=128))
            ident16 = wpool.tile([128, 128], f16)
            make_identity(nc, ident16[:])
            # dummy exp: pulls the 1.3us ACT_TABLE_LOAD into the startup
            # window instead of the first real exp
            scr = wpool.tile([1, 2], f16)
            nc.scalar.activation(scr[:], bo_sb[0:1, 0:2], AF.Exp)



            # ---- phase 1: accumulate per-head [num | den] over local tokens ----
            # cm_ps[hp] rows 0:64   = head 2hp   : cols 0:64 num, col 64 den
            #           rows 64:128 = head 2hp+1 : cols 65:129 num, col 129 den
            xbig = x16pool.tile([128, 2, T], f16, name="xbig")

            with (
                tc.tile_pool(name="ps_cm", bufs=1, space="PSUM") as ps_cm,
                tc.tile_pool(name="ps_kv", bufs=1, space="PSUM") as ps_kv,
            ):
                cm_ps = [
                    ps_cm.tile([128, 130], f32, tag=f"cm{i}", name=f"cm{i}")
                    for i in range(4)
                ]
                ctx_next = ctx_first
                pend = None  # (kexp2, vcat2, is_first) awaiting its cm matmuls
                def flush_cm(stop):
                    kexp2, vcat2, first = pend
                    for hp in range(4):
                        nc.tensor.matmul(
                            cm_ps[hp][:],
                            lhsT=kexp2[:, :, hp * 128 : (hp + 1) * 128],
                            rhs=vcat2[:, :, 2 * hp : 2 * hp + 2, :],
                            start=first,
                            stop=stop,
                            perf_mode=DR,
                        )
                for ci in range(NCH):
                    ctx_t = ctx_next
                    if ci + 1 < NCH:
                        ctx_next = spool.tile(
                            [128, 2, CHUNK], f8, tag="ctx", name=f"ctx{ci+1}"
                        )
                        nc.sync.dma_start(
                            ctx_next[:], ch_r[:, :, (ci + 1) * CHUNK : (ci + 2) * CHUNK]
                        )
                    # phase-2 x slice: direct fp16 load into the big tile
                    nc.sync.dma_start(
                        xbig[:, :, ci * CHUNK : (ci + 1) * CHUNK],
                        xh_r[:, :, ci * CHUNK : (ci + 1) * CHUNK],
                    )

                    for p in range(PAIRS):
                        # two 128-token subtiles per iteration (DoubleRow cm)
                        k2_ps = ps_kv.tile([128, 2, INNER], f32, tag="k2")
                        for s in range(2):
                            tok = slice((2 * p + s) * 128, (2 * p + s + 1) * 128)
                            nc.tensor.matmul(
                                k2_ps[:, s, :],
                                lhsT=ctx_t[:, :, tok],
                                rhs=wk8[:],
                                start=True,
                                stop=True,
                                perf_mode=DR,
                            )
                        # batched exp over both subtiles; un-scale the 16x
                        # weight scaling inside the activation
                        kexp2 = spool.tile([128, 2, INNER], f8, tag="kexp")
                        nc.scalar.activation(kexp2[:], k2_ps[:], AF.Exp, scale=0.0625)
                        v2_ps = ps_kv.tile([128, 2, INNER], f32, tag="v2")
                        for s in range(2):
                            tok = slice((2 * p + s) * 128, (2 * p + s + 1) * 128)
                            nc.tensor.matmul(
                                v2_ps[:, s, :],
                                lhsT=ctx_t[:, :, tok],
                                rhs=wv8[:],
                                start=True,
                                stop=True,
                                perf_mode=DR,
                            )
                        vcat2 = spool.tile([128, 2, 8, 65], f8, tag="vcat")
                        vsrc = v2_ps[:].rearrange("q s (h e) -> q s h e", h=8)
                        nc.vector.tensor_copy(vcat2[:, :, :, 0:64], vsrc)
                        # den column: 16.0 matches the 16x scale baked into wv.
                        # On DVE (not gpsimd) so the gpsimd queue is empty when
                        # the collective trigger lands on it.
                        nc.vector.memset(vcat2[:, :, :, 64], 16.0)
                        # software pipelining: cm matmuls run one pair behind,
                        # so the PE never stalls waiting for this pair's
                        # exp/vcat to drain
                        if pend is not None:
                            flush_cm(False)
                        pend = (kexp2, vcat2, ci == 0 and p == 0)
                flush_cm(True)

                # ---- stage [num|den] for the AllReduce (PSUM -> SBUF -> DRAM) ----
                cm_sb = wpool.tile([128, 4, 65], f32)
                for hp in range(4):
                    nc.vector.tensor_copy(cm_sb[0:64, hp, :], cm_ps[hp][0:64, 0:65])
                    nc.vector.tensor_copy(
                        cm_sb[64:128, hp, :], cm_ps[hp][64:128, 65:130]
                    )
                cc_in = dpool.tile([128, 4, 65], f32)
                nc.sync.dma_start(cc_in[:], cm_sb[:])

            cc_out = dpool.tile([128, 4, 65], f32)
            nc.gpsimd.collective_compute(
                "AllReduce",
                mybir.AluOpType.add,
                replica_groups=[[0, 1], [2, 3], [4, 5], [6, 7]],
                ins=[cc_in.opt()],
                outs=[cc_out.opt()],
            )
            mm_sb = wpool.tile([128, 4, 65], f32)
            nc.sync.dma_start(mm_sb[:], cc_out[:])

            with tc.tile_pool(name="ps_p2", bufs=1, space="PSUM") as ps_p2:
                # keep the PE p-state up through the AllReduce window
                # (fine-grained so real work drains the queue quickly)
                warm_ps = ps_p2.tile([128, INNER], f32, tag="o", name="warm_ps", bufs=1)
                for _ in range(150):
                    nc.tensor.matmul(
                        warm_ps[:, 0:128],
                        lhsT=wqt16[:, 0, 0:128],
                        rhs=wqt16[:, 1, 0:128],
                        start=True,
                        stop=True,
                    )

                # ---- normalize cm, build W_eff = I + sum_h Wq_h cm_h Wo_h ----
                deninv = wpool.tile([128, 4], f32)
                cmn16 = wpool.tile([128, 4, 64], f16)
                m1t16 = wpool.tile([64, 8, C], f16)
                weff16 = wpool.tile([128, 2, C], f16)
                with tc.tile_pool(name="ps_post", bufs=1, space="PSUM") as ps_post:
                    nc.vector.reciprocal(deninv[:], mm_sb[:, :, 64])
                    for hp in range(4):
                        nc.vector.tensor_scalar_mul(
                            cmn16[:, hp, :],
                            mm_sb[:, hp, 0:64],
                            deninv[:, hp : hp + 1],
                        )
                    for h in range(HEADS):
                        hp, hh = h // 2, h % 2
                        rs = slice(hh * 64, hh * 64 + 64)
                        m1t_ps = ps_post.tile([64, C], f32, tag="m1t", bufs=2)
                        nc.tensor.matmul(
                            m1t_ps[:],
                            lhsT=cmn16[rs, hp, :],
                            rhs=wqt16[rs, hp, :],
                            start=True,
                            stop=True,
                        )
                        nc.vector.tensor_copy(m1t16[:, h, :], m1t_ps[:])
                    for ic in range(2):
                        weff_ps = ps_post.tile([128, C], f32, tag="weff")
                        for h in range(HEADS):
                            nc.tensor.matmul(
                                weff_ps[:],
                                lhsT=m1t16[:, h, ic * 128 : (ic + 1) * 128],
                                rhs=wo16[:, h, :],
                                start=(h == 0),
                                stop=False,
                            )
                        # fold the residual in: W_eff += I (this core's row block)
                        nc.tensor.matmul(
                            weff_ps[:, ic * 128 : (ic + 1) * 128],
                            lhsT=ident16[:],
                            rhs=ident16[:],
                            start=False,
                            stop=True,
                        )
                        nc.vector.tensor_copy(weff16[:, ic, :], weff_ps[:])

                # ---- phase 2: out = W_eff^T @ x (+bo), fp16 out ----
                # 1024-wide matmuls (2-bank PSUM tiles) halve the LDWEIGHTS
                # count; stores issue per-oc so the out DMA drains early
                G = 1024
                for g in range(T // G):
                    ts_ = slice(g * G, (g + 1) * G)
                    out_sb = spool.tile([128, 2, G], f16, tag="out_sb")
                    for oc in range(2):
                        o_ps = ps_p2.tile([128, G], f32, tag="o2", bufs=2)
                        for nh in range(2):
                            for ic in range(2):
                                nc.tensor.matmul(
                                    o_ps[:, nh * 512 : (nh + 1) * 512],
                                    lhsT=weff16[:, ic, oc * 128 : (oc + 1) * 128],
                                    rhs=xbig[
                                        :, ic, g * G + nh * 512 : g * G + (nh + 1) * 512
                                    ],
                                    start=(ic == 0),
                                    stop=(ic == 1),
                                )
                        # bias+copy: alternate ACT / DVE to balance engines
                        if (g + oc) % 2 == 0:
                            nc.scalar.activation(
                                out_sb[:, oc, :],
                                o_ps[:],
                                AF.Identity,
                                bias=bo_sb[:, oc : oc + 1],
                            )
                        else:
                            nc.vector.tensor_scalar_add(
                                out_sb[:, oc, :], o_ps[:], bo_sb[:, oc : oc + 1]
                            )
                        nc.sync.dma_start(
                            out_r[:, oc, ts_], out_sb[:, oc, :]
                        )

    nc.compile()
    return nc


def _get_nc():
    if "nc" not in _CACHE:
        _CACHE["nc"] = _build_nc()
    return _CACHE["nc"]


def kernel(**inputs) -> np.ndarray:
    global LAST_RESULTS
    import ml_dtypes
    from concourse.bass_utils import run_bass_kernel_spmd

    f8 = ml_dtypes.float8_e4m3fn
    x = np.asarray(inputs["x"], dtype=np.float32)
    ctx = np.asarray(inputs["context"], dtype=np.float32)
    Wq = np.asarray(inputs["Wq"], dtype=np.float32)
    Wk = np.asarray(inputs["Wk"], dtype=np.float32)
    Wv = np.asarray(inputs["Wv"], dtype=np.float32)
    Wo = np.asarray(inputs["Wo"], dtype=np.float32)
    bo = np.ascontiguousarray(
        np.asarray(inputs["bo"], dtype=np.float32).reshape(C, 1)
    )
    wqt = np.ascontiguousarray(Wq.T.astype(np.float16))
    wo16 = np.ascontiguousarray(Wo.astype(np.float16))
    wk8 = np.ascontiguousarray((Wk * 16.0).astype(f8))
    wv8 = np.ascontiguousarray((Wv * 16.0).astype(f8))

    xf = x.reshape(B, C, N_FULL).astype(np.float16)
    cf = ctx.reshape(B, C, N_FULL).astype(f8)

    in_maps = []
    for c in range(NCORES):
        b, t = c // 2, c % 2
        sl = slice(t * T, (t + 1) * T)
        in_maps.append(
            {
                "xh": np.ascontiguousarray(xf[b, :, sl]),
                "ch": np.ascontiguousarray(cf[b, :, sl]),
                "wk": wk8,
                "wv": wv8,
                "wqt": wqt,
                "wo": wo16,
                "bo": bo,
            }
        )

    nc = _get_nc()
    res = run_bass_kernel_spmd(nc, in_maps, list(range(NCORES)), trace=TRACE)
    LAST_RESULTS = res

    out = np.empty((B, C, N_FULL), dtype=np.float32)
    for c in range(NCORES):
        b, t = c // 2, c % 2
        out[b, :, t * T : (t + 1) * T] = res.results[c]["out"].astype(np.float32)
    return out.reshape(B, C, 128, 128)
